# revision 37
# baseline (speedup 1.0000x reference)
"""Trainium2 Bass kernel for GQA attention (QK-RMSNorm + RoPE + softmax + o_proj).

Sharding over 8 NeuronCores: core = (batch b in {0,1}, sequence quarter sc in
{0..3}).  Each core produces the final output rows for its 512 queries:
  - K/V projections for its own 512 rows (all 4 kv heads), RMSNorm+RoPE on K,
  - AllGather of K^T/V across the 4 cores of the same batch -> full-S K/V,
  - Q projection heads 0-7 + RMSNorm + RoPE (overlaps the AllGather);
    heads 8-15 are projected INSIDE attention groups 0-3 (2 matmuls per
    t-step, weights streamed 4KB/head) so attention starts ~40us earlier
    and the PE slack under the Act-bound exp chain is used,
  - scores^T tiles = K^T_tile.T @ Q^T, exp on the Act engine,
  - row-sums via dual DVE accumulators + GPSIMD partition all-reduce,
  - AV accumulation in PSUM, 1/sum scale,
  - o_proj split: heads 0-7 interleaved into attention groups 4-7 (PSUM
    psc:4 + pav:2 + po:2 banks), heads 8-15 after, merged via one add.

v2 changes vs the first working version:
  - DMAs are batched with multi-dim access patterns (rearranged DRAM APs):
    ~25 DMA instructions instead of ~145.  The HWDGE dispatch cost (~0.6us
    per DMA, serialized) made the DMA skeleton alone 141us/iter on HW.
  - RMSNorm weights (q_norm_w/k_norm_w) are folded into the RoPE cos/sin
    tables on the host, removing one elementwise mul per head.
  - rsqrt is computed as exp(-0.5*ln(ms+eps)) so every Act-engine function
    (Ln/Exp/Square/Copy) lives in ONE activation table -> no table swaps.
  - K^T readback uses an r-major column layout so the whole gather is one
    DMA descriptor set.

All matmul operands are bfloat16 (PE full rate); accumulation stays f32 in
PSUM.  fp8 was analyzed and rejected: softmax amplifies q/k quantization
error (~5% per element) into ~7% attention-weight error, far over the 2e-2
budget.

kernel(**inputs) takes FULL unsharded inputs, returns the full output.
Host-side prep (weight transposes, table folding, bf16 cast) is numpy.
"""
import contextlib

import ml_dtypes
import numpy as np

import concourse.bass as bass
import concourse.bass_isa as bass_isa
import concourse.mybir as mybir
import concourse.tile as tile
from concourse import bacc
from concourse.bass_utils import run_bass_kernel_spmd

B, S, HID = 2, 2048, 2048
NH, NKV, D = 16, 4, 128
SC = 512           # per-core sequence chunk (queries)
KT_H = HID // 128  # 16 contraction tiles over hidden dim
EPS = 1e-6
INV_SQRT_D = 1.0 / float(np.sqrt(D))

BF16 = mybir.dt.bfloat16
F32 = mybir.dt.float32
NP_BF16 = ml_dtypes.bfloat16

# rope table column offsets in the packed ropeT tensor [D, 4*SC]
COSQ, SINQ, COSK, SINK = 0, SC, 2 * SC, 3 * SC

ACT = mybir.ActivationFunctionType


def _pin_act_table():
    """Every Act function this kernel uses (Copy/Square/Ln/Exp) lives in the
    natural_log_exp_and_others table, but the auto table-load pass greedily
    first-matches each function (Ln -> natural_log, Exp -> exp_and_others),
    inserting ~37 table swaps (~1.3us each).  Emptying the other tables in
    the cached dict makes every function resolve to the one shared table
    (its index — what walrus consumes — is unchanged), so exactly one load
    is emitted."""
    from concourse.hw_specs import get_activation_tables
    tabs = get_activation_tables("gen3")
    for k, s in tabs.items():
        if k != "natural_log_exp_and_others":
            s.clear()


def build_nc(mode="real", max_iters=64, upto="full"):
    """mode: 'real' (with AllGather), 'timed' (AllGather emulated by local
    DMA copies, body wrapped in a runtime-count For_i hardware loop), or
    'sim' (AllGather emulated, no loop — for TimelineSim)."""
    _pin_act_table()
    nc = bacc.Bacc("TRN2", target_bir_lowering=False, debug=False, num_devices=8)

    d = {}
    d["xT"] = nc.dram_tensor("xT", [HID, SC], BF16, kind="ExternalInput")
    d["wqT"] = nc.dram_tensor("wqT", [HID, NH * D], BF16, kind="ExternalInput")
    d["wkT"] = nc.dram_tensor("wkT", [HID, NKV * D], BF16, kind="ExternalInput")
    d["wvT"] = nc.dram_tensor("wvT", [HID, NKV * D], BF16, kind="ExternalInput")
    d["woT"] = nc.dram_tensor("woT", [NH * D, HID], BF16, kind="ExternalInput")
    d["ropeT"] = nc.dram_tensor("ropeT", [D, 4 * SC], BF16, kind="ExternalInput")
    d["out"] = nc.dram_tensor("out", [SC, HID], F32, kind="ExternalOutput")
    if mode == "timed":
        d["nit"] = nc.dram_tensor("nit", [1, 1], mybir.dt.int32, kind="ExternalInput")

    d["kv_local"] = nc.dram_tensor("kv_local", [SC, 1024], BF16)
    d["kv_all"] = nc.dram_tensor("kv_all", [4 * SC, 1024], BF16)

    with tile.TileContext(nc) as tc, \
         nc.allow_low_precision(reason="bf16 operands, f32 accumulation"):
        with contextlib.ExitStack() as ctx:
            cpool = ctx.enter_context(tc.tile_pool(name="consts", bufs=1))
            qt_pool = ctx.enter_context(tc.tile_pool(name="qt", bufs=1))

            ropes = cpool.tile([128, 4 * SC], BF16, name="ropes")
            nc.scalar.dma_start(out=ropes[:], in_=d["ropeT"][:])
            epsc = cpool.tile([128, 1], F32, name="epsc")
            nc.gpsimd.memset(epsc[:], EPS)

            QT = qt_pool.tile([128, NH * SC], BF16)  # [d, h*SC + sq]
            # K/V readback tiles live in the outer pool: their SBUF must be
            # disjoint from the proj-phase tiles, otherwise the readback DMA
            # picks up a write-after-read dependency on the whole Q
            # projection (readback would stall until the proj pools' last
            # reader).
            KTs = qt_pool.tile([128, NKV * S], BF16, tag="KTs", name="KTs")
            Vs = qt_pool.tile([128, NKV * S], BF16, tag="Vs", name="Vs")

            if mode == "timed":
                nit_sb = cpool.tile([1, 1], mybir.dt.int32)
                nc.sync.dma_start(out=nit_sb[:], in_=d["nit"][:])
                with tc.tile_critical():
                    regs = []
                    for e in mybir.ALL_ENGINES:
                        eng = nc.engines[e]
                        tmp = eng.alloc_register(f"nit_{e.name}")
                        eng.reg_load(tmp, nit_sb[0:1, 0:1])
                        regs.append(tmp)
                    n_val = nc.snap(bass.RegisterHandles(regs), donate=True,
                                    min_val=0, max_val=max_iters)
                loop_cm = tc.For_i(0, n_val, 1)
            else:
                loop_cm = contextlib.nullcontext()

            with loop_cm:
                _emit_body(nc, tc, mode, d, ropes, epsc, QT, KTs, Vs, upto)
                if mode == "timed":
                    dummy = cpool.tile([1, 8], F32)
                    nc.gpsimd.memset(dummy[:], 0.0)

    nc.compile()
    return nc


def _rope_stage1(nc, rope_pool, x_psum, copy_eng=None):
    """Stage 1 of RMSNorm+RoPE: copy PSUM -> bf16 (the bank's ONLY reader,
    so it frees immediately), square on DVE, partition all-reduce on GPSIMD.
    Returns state for _rope_stage2.  copy_eng overrides the copy engine —
    the last Q heads use DVE so the PSUM banks the attention scores reuse
    are not released behind the Act queue's rope tail."""
    xb = rope_pool.tile([128, SC], BF16, tag="xb", bufs=3, name="xb")
    if copy_eng is None:
        nc.scalar.copy(xb[:], x_psum[:])
    else:
        copy_eng.tensor_copy(xb[:], x_psum[:])
    sq = rope_pool.tile([128, SC], BF16, tag="sq", bufs=2, name="sq")
    nc.vector.tensor_mul(sq[:], xb[:], xb[:])
    pssum = rope_pool.tile([128, SC], F32, tag="pss", bufs=2, name="pssum")
    nc.gpsimd.partition_all_reduce(pssum[:], sq[:], channels=128,
                                   reduce_op=bass_isa.ReduceOp.add)
    return xb, pssum


def _rope_stage2(nc, rope_pool, ropes, epsc, state, dst, dst_col, cos_col,
                 sin_col):
    """Stage 2: rinv = exp(-0.5*ln(ms+eps)) on Act (both functions live in
    the natural_log_exp table -> no table reloads), then the two
    partition-shifted sin muls + cos mul + add + rinv mul on DVE.

    Callers pipeline stage2 of head h-1 behind stage1 of head h, so the Act
    queue never stalls on a fresh all-reduce and the PSUM bank release
    (stage1's copy) is not queued behind ln/exp of earlier heads."""
    xb, pssum = state
    lns = rope_pool.tile([128, SC], BF16, tag="ln", bufs=2, name="lns")
    nc.scalar.activation(lns[:], pssum[:], ACT.Ln, bias=epsc[:], scale=1.0 / D)
    rinv = rope_pool.tile([128, SC], BF16, tag="rin", bufs=2, name="rinv")
    nc.scalar.activation(rinv[:], lns[:], ACT.Exp, bias=0.0, scale=-0.5)
    a_t = rope_pool.tile([128, SC], BF16, tag="a", bufs=2, name="a_t")
    nc.vector.tensor_mul(a_t[:], xb[:], ropes[:, cos_col:cos_col + SC])
    # b[0:64] = x[64:128]*sinSh[64:128]; b[64:128] = x[0:64]*sinSh[0:64]
    # (sin table is rolled by 64 along d on the host so both INPUTS share
    # the same partition base; only out is shifted.)
    b_t = rope_pool.tile([128, SC], BF16, tag="b", bufs=2, name="b_t")
    nc.vector.tensor_mul(b_t[0:64, :], xb[64:128, :],
                         ropes[64:128, sin_col:sin_col + SC])
    nc.vector.tensor_mul(b_t[64:128, :], xb[0:64, :],
                         ropes[0:64, sin_col:sin_col + SC])
    ab_t = rope_pool.tile([128, SC], BF16, tag="ab", bufs=2, name="ab_t")
    nc.vector.tensor_add(ab_t[:], a_t[:], b_t[:])
    nc.vector.tensor_mul(dst[:, dst_col:dst_col + SC], ab_t[:], rinv[:])


def _emit_loads(nc, d, hsT, wk_full, wv_full, wq07):
    """Batched input loads.  sync queue: wk/hs interleaved so the K
    projection's kt-chain is never more than one chunk behind the DMA
    stream; then wv and the heads-0..7 slice of wq (heads 8..15 stream
    during attention)."""
    wkr = d["wkT"][:].rearrange("(k p) n -> p k n", p=128)  # (128, 16, 512)
    hsr = d["xT"][:].rearrange("(k p) s -> p k s", p=128)
    wvr = d["wvT"][:].rearrange("(k p) n -> p k n", p=128)
    wk3 = wk_full[:].rearrange("p (k n) -> p k n", n=512)
    hs3 = hsT[:].rearrange("p (k s) -> p k s", s=512)
    wv3 = wv_full[:].rearrange("p (k n) -> p k n", n=512)
    bounds = [0, 1, 2, 4, 8, 12, 16]
    for c in range(6):
        lo, hi = bounds[c], bounds[c + 1]
        nc.sync.dma_start(out=wk3[:, lo:hi, :], in_=wkr[:, lo:hi, :])
        nc.sync.dma_start(out=hs3[:, lo:hi, :], in_=hsr[:, lo:hi, :])
    nc.sync.dma_start(out=wv3[:, 0:8, :], in_=wvr[:, 0:8, :])
    nc.sync.dma_start(out=wv3[:, 8:16, :], in_=wvr[:, 8:16, :])
    wqr = d["wqT"][:].rearrange("(k p) n -> p k n", p=128)  # (128, 16, 2048)
    wq3 = wq07[:].rearrange("p (k n) -> p k n", n=1024)
    nc.sync.dma_start(out=wq3[:, 0:8, :], in_=wqr[:, 0:8, 0:1024])
    nc.sync.dma_start(out=wq3[:, 8:16, :], in_=wqr[:, 8:16, 0:1024])


def _emit_body(nc, tc, mode, d, ropes, epsc, QT, KTs, Vs, upto="full"):
    if upto == "dma":
        _emit_dma_only(nc, tc, mode, d, ropes, KTs, Vs)
        return
    # ---------------- projections ----------------
    # Pool stack (LIFO): octx{avt} -> mid{hsT,rope,wqh} -> early{wkv,...}
    # -> attention pools -> (close attention, close mid) -> B pools in octx.
    # mid stays open into the attention phase: Q heads 8..15 are projected
    # *inside* attention groups 0-3 (2 matmuls per t-step) to fill the PE
    # slack under the Act-bound exp chain and start attention ~40us earlier.
    octx = contextlib.ExitStack()
    avt_pool = octx.enter_context(tc.tile_pool(name="avt", bufs=1))
    AVT = avt_pool.tile([128, NH * SC], BF16)  # [dv, h*SC + sq]
    octA = avt_pool.tile([128, 16 * 512], BF16, name="octA")
    mid_ctx = octx.enter_context(contextlib.ExitStack())
    hs_pool = mid_ctx.enter_context(tc.tile_pool(name="hsT", bufs=1))
    rope_pool = mid_ctx.enter_context(tc.tile_pool(name="rope", bufs=1))
    wqh_pool = mid_ctx.enter_context(tc.tile_pool(name="wqh", bufs=1))
    hsT = hs_pool.tile([128, KT_H * SC], BF16)

    with contextlib.ExitStack() as ctx:
        w_pool = ctx.enter_context(tc.tile_pool(name="wkv", bufs=1))
        st_pool = ctx.enter_context(tc.tile_pool(name="kvst", bufs=1))
        pp = ctx.enter_context(tc.tile_pool(name="pproj", bufs=1, space="PSUM"))

        wk_full = w_pool.tile([128, KT_H * NKV * D], BF16)
        wv_full = w_pool.tile([128, KT_H * NKV * D], BF16)
        wq07 = w_pool.tile([128, KT_H * 1024], BF16)
        _emit_loads(nc, d, hsT, wk_full, wv_full, wq07)

        kst = st_pool.tile([128, NKV * SC], BF16, name="kst")
        vst = st_pool.tile([128, NKV * SC], BF16, name="vst")

        # K proj + norm/rope -> kst[:, kvh*SC:+SC]; one batched store.
        # rope stages are software-pipelined across heads (see _rope_stage2).
        kstate = None
        for kvh in range(NKV):
            psk = pp.tile([128, SC], F32, tag="pq", bufs=4, name="psk")
            for kt in range(KT_H):
                nc.tensor.matmul(
                    psk[:],
                    wk_full[:, kt * 512 + kvh * D: kt * 512 + (kvh + 1) * D],
                    hsT[:, kt * SC:(kt + 1) * SC],
                    start=(kt == 0), stop=(kt == KT_H - 1))
            st1 = _rope_stage1(nc, rope_pool, psk)
            if kstate is not None:
                _rope_stage2(nc, rope_pool, ropes, epsc, kstate[0], kst,
                             kstate[1] * SC, COSK, SINK)
            kstate = (st1, kvh)
        _rope_stage2(nc, rope_pool, ropes, epsc, kstate[0], kst,
                     kstate[1] * SC, COSK, SINK)
        nc.sync.dma_start(
            out=d["kv_local"][:, 0:512].rearrange("(kvh d) s -> d kvh s", d=128),
            in_=kst[:].rearrange("d (kvh s) -> d kvh s", s=512))
        if mode != "real":
            # K-column AllGather emulation right after the K store: its
            # only dependency is kst, so the K^T readback lands ~13us
            # earlier than with full-row copies gated on V too.
            for r in range(4):
                nc.sync.dma_start(
                    out=d["kv_all"][r * SC:(r + 1) * SC, 0:512],
                    in_=d["kv_local"][:, 0:512])
            nc.sync.dma_start(
                out=KTs[:].rearrange("d (r kvh s) -> d r kvh s", kvh=4, s=512),
                in_=d["kv_all"][:, 0:512].rearrange(
                    "(r kvh d) s -> d r kvh s", r=4, kvh=4))

        # V proj -> vst ([seq, dv] layout, all 4 kv heads); one store.
        for st in range(4):
            psv = pp.tile([128, SC], F32, tag="pq", bufs=4, name="psv")
            for kt in range(KT_H):
                nc.tensor.matmul(
                    psv[:],
                    hsT[:, kt * SC + st * 128: kt * SC + (st + 1) * 128],
                    wv_full[:, kt * 512:(kt + 1) * 512],
                    start=(kt == 0), stop=(kt == KT_H - 1))
            nc.scalar.copy(vst[:, st * SC:(st + 1) * SC], psv[:])
        nc.scalar.dma_start(
            out=d["kv_local"][:, 512:1024].rearrange("(st p) c -> p st c", p=128),
            in_=vst[:].rearrange("p (st c) -> p st c", c=512))

        # streamed wq tiles for deferred heads 8..15 (4KB each, bufs=3:
        # tile dh's DMA waits for head dh-3's last matmul, pacing itself
        # ahead of use).  The first two interleave between the Vs halves
        # so every attention-start gate clears before the PE arrives.
        wqr2 = d["wqT"][:].rearrange("(k p) n -> p k n", p=128)
        wqh_t = []
        for dh in range(8):
            wqh_t.append(wqh_pool.tile([128, KT_H * D], BF16, tag="wqh",
                                       bufs=3, name=f"wqh{dh}"))

        def wqh_load(dh):
            nc.sync.dma_start(
                out=wqh_t[dh][:].rearrange("p (k n) -> p k n", n=128),
                in_=wqr2[:, :, (8 + dh) * 128:(9 + dh) * 128])

        Vs4 = Vs[:].rearrange("p (r tt c) -> p r tt c", tt=4, c=512)
        kva4 = d["kv_all"][:, 512:1024].rearrange(
            "(r tt p) c -> p r tt c", r=4, tt=4)
        if mode == "real":
            nc.gpsimd.collective_compute(
                "AllGather", mybir.AluOpType.bypass,
                ins=[d["kv_local"][:]], outs=[d["kv_all"][:]],
                replica_groups=[[0, 1, 2, 3], [4, 5, 6, 7]])
            nc.sync.dma_start(
                out=KTs[:].rearrange("d (r kvh s) -> d r kvh s", kvh=4, s=512),
                in_=d["kv_all"][:, 0:512].rearrange(
                    "(r kvh d) s -> d r kvh s", r=4, kvh=4))
            nc.sync.dma_start(out=Vs4[:], in_=kva4[:])
            for dh in range(2):
                wqh_load(dh)
        else:
            # V-column AllGather emulation, then Vs halves with the first
            # two streamed-wq tiles interleaved.
            for r in range(4):
                nc.sync.dma_start(
                    out=d["kv_all"][r * SC:(r + 1) * SC, 512:1024],
                    in_=d["kv_local"][:, 512:1024])
            nc.sync.dma_start(out=Vs4[:, 0:2, :, :], in_=kva4[:, 0:2, :, :])
            wqh_load(0)
            wqh_load(1)
            nc.sync.dma_start(out=Vs4[:, 2:4, :, :], in_=kva4[:, 2:4, :, :])

        # Q proj heads 0..7 + norm/rope -> QT (overlaps the AllGather)
        qstate = None
        for h in range(8):
            psq = pp.tile([128, SC], F32, tag="pq", bufs=4, name="psq")
            for kt in range(KT_H):
                nc.tensor.matmul(
                    psq[:],
                    wq07[:, kt * 1024 + h * D: kt * 1024 + (h + 1) * D],
                    hsT[:, kt * SC:(kt + 1) * SC],
                    start=(kt == 0), stop=(kt == KT_H - 1))
            st1 = _rope_stage1(nc, rope_pool, psq)
            if qstate is not None:
                _rope_stage2(nc, rope_pool, ropes, epsc, qstate[0], QT,
                             qstate[1] * SC, COSQ, SINQ)
            qstate = (st1, h)
        _rope_stage2(nc, rope_pool, ropes, epsc, qstate[0], QT,
                     qstate[1] * SC, COSQ, SINQ)

    if upto == "proj":
        mid_ctx.close()
        with tc.tile_pool(name="fin", bufs=1) as fin:
            ft = fin.tile([128, 512], F32)
            nc.vector.tensor_copy(ft[:], QT[:, 0:512])
            nc.sync.dma_start(out=d["out"][0:128, 0:512], in_=ft[:])
        return

    # ---------------- attention ----------------
    with contextlib.ExitStack() as ctx:
        wo_pool = ctx.enter_context(tc.tile_pool(name="wo", bufs=1))
        pt_pool = ctx.enter_context(tc.tile_pool(name="pt", bufs=1))
        sm_pool = ctx.enter_context(tc.tile_pool(name="sm", bufs=1))
        pa = ctx.enter_context(tc.tile_pool(name="pattn", bufs=1, space="PSUM"))

        # o_proj weights, first half (heads 0..7) — used by the A slots.
        # Second half streams in after the mid pools close (group 3).
        wo_full = wo_pool.tile([128, 8 * HID], BF16)
        wor = d["woT"][:].rearrange("(j p) h -> p j h", p=128)
        wo3 = wo_full[:].rearrange("p (j h) -> p j h", h=2048)
        for w in range(4):
            nc.gpsimd.dma_start(out=wo3[:, w * 2:(w + 1) * 2, :],
                                in_=wor[:, w * 2:(w + 1) * 2, :])

        # remaining streamed wq tiles (2..7); bufs=3 rotation paces them.
        for dh in range(2, 8):
            wqh_load(dh)

        cur_po = [None]

        def oproj_a_slot(s):
            """o_proj half-A matmul for global slot s (0..127)."""
            tile_idx, j = divmod(s, 8)
            st, hc = divmod(tile_idx, 4)
            if j == 0:
                cur_po[0] = pa.tile([128, 512], F32, tag="po", bufs=2,
                                    name="poA")
            po = cur_po[0]
            nc.tensor.matmul(
                po[:],
                AVT[:, j * SC + st * 128: j * SC + (st + 1) * 128],
                wo_full[:, j * HID + hc * 512: j * HID + (hc + 1) * 512],
                start=(j == 0), stop=(j == 7), skip_group_check=True)
            if j == 7:
                nc.scalar.copy(octA[:, tile_idx * 512:(tile_idx + 1) * 512],
                               po[:])

        cur_psq = [None]
        dqstate = [qstate]

        def qproj_slot(s):
            """Deferred Q-proj matmul for slot s (0..127): head 8 + s//16,
            contraction step s%16.  Shares the "po" PSUM tag with the A
            slots (Q slots end at group 3, A slots start at group 4)."""
            dh, kt = divmod(s, 16)
            if kt == 0:
                cur_psq[0] = pa.tile([128, SC], F32, tag="po", bufs=2,
                                     name="psqd")
            psq = cur_psq[0]
            nc.tensor.matmul(
                psq[:],
                wqh_t[dh][:, kt * D:(kt + 1) * D],
                hsT[:, kt * SC:(kt + 1) * SC],
                start=(kt == 0), stop=(kt == KT_H - 1), skip_group_check=True)
            if kt == KT_H - 1:
                st1 = _rope_stage1(nc, rope_pool, psq, copy_eng=nc.vector)
                prev = dqstate[0]
                _rope_stage2(nc, rope_pool, ropes, epsc, prev[0], QT,
                             prev[1] * SC, COSQ, SINQ)
                dqstate[0] = (st1, 8 + dh)

        # heads in pairs sharing the kv head: score/AV matmuls share the
        # stationary operand; exp + row-sum accumulation run batched over
        # the pair ([128, 1024] tiles).
        for grp in range(NH // 2):
            kvh = grp // 2
            h0 = 2 * grp
            pav = [pa.tile([128, SC], F32, tag=f"pavt{j}", bufs=1,
                           name=f"pav{j}") for j in range(2)]
            ptacc = [sm_pool.tile([128, 2 * SC], BF16, tag=f"ptacc{p}",
                                  bufs=2, name=f"ptacc{p}") for p in range(2)]

            def kcol(t):
                r, tt = divmod(t, 4)
                return r * 2048 + kvh * 512 + tt * 128

            def emit_score(t):
                psc = pa.tile([128, 2 * SC], F32, tag="psc", bufs=2, name="psc")
                for j in range(2):
                    nc.tensor.matmul(
                        psc[:, j * SC:(j + 1) * SC],
                        KTs[:, kcol(t): kcol(t) + 128],
                        QT[:, (h0 + j) * SC:(h0 + j + 1) * SC],
                        start=True, stop=True)
                pt_t = pt_pool.tile([128, 2 * SC], BF16, tag="pt", bufs=3,
                                    name="pt_t")
                nc.scalar.activation(pt_t[:], psc[:], ACT.Exp,
                                     bias=0.0, scale=INV_SQRT_D)
                return pt_t

            pts = emit_score(0)
            for t in range(16):
                pt_t = pts
                pts = emit_score(t + 1) if t < 15 else None
                for j in range(2):
                    nc.tensor.matmul(
                        pav[j][:],
                        Vs[:, t * 512 + kvh * D: t * 512 + (kvh + 1) * D],
                        pt_t[:, j * SC:(j + 1) * SC],
                        start=(t == 0), stop=(t == 15), skip_group_check=True)
                if grp < 4:
                    base = (grp * 16 + t) * 2
                    qproj_slot(base)
                    qproj_slot(base + 1)
                else:
                    base = ((grp - 4) * 16 + t) * 2
                    oproj_a_slot(base)
                    oproj_a_slot(base + 1)
                pta = ptacc[t % 2]
                if t < 2:
                    nc.vector.tensor_copy(pta[:], pt_t[:])
                else:
                    nc.vector.tensor_add(pta[:], pta[:], pt_t[:])
            # free the pav banks fast (see v2 notes), then the softmax
            # denominator chain off the PE critical path.
            pavb = [sm_pool.tile([128, SC], BF16, tag=f"pavb{j}", bufs=2,
                                 name=f"pavb{j}") for j in range(2)]
            for j in range(2):
                nc.vector.tensor_copy(pavb[j][:], pav[j][:])
            ptsum = sm_pool.tile([128, 2 * SC], BF16, tag="ptsum", bufs=2,
                                 name="ptsum")
            nc.vector.tensor_add(ptsum[:], ptacc[0][:], ptacc[1][:])
            zb = sm_pool.tile([128, 2 * SC], F32, tag="zb", bufs=2, name="zb")
            nc.gpsimd.partition_all_reduce(zb[:], ptsum[:], channels=128,
                                           reduce_op=bass_isa.ReduceOp.add)
            rz = sm_pool.tile([128, 2 * SC], BF16, tag="rz", bufs=2, name="rz")
            nc.vector.reciprocal(rz[:], zb[:])
            for j in range(2):
                h = h0 + j
                nc.vector.tensor_mul(AVT[:, h * SC:(h + 1) * SC], pavb[j][:],
                                     rz[:, j * SC:(j + 1) * SC])

            if grp == 3:
                # last deferred head's rope (stage2 uses the mid rope pool)
                prev = dqstate[0]
                _rope_stage2(nc, rope_pool, ropes, epsc, prev[0], QT,
                             prev[1] * SC, COSQ, SINQ)

    # attention pools closed; free the mid pools (hsT, rope temps, wq
    # stream) so the second half of the o_proj weights fits.
    mid_ctx.close()

    if upto == "attn":
        with tc.tile_pool(name="fin", bufs=1) as fin:
            ft = fin.tile([128, 512], F32)
            nc.vector.memset(ft[:], 0.0)
            nc.sync.dma_start(out=d["out"][0:128, 0:512], in_=ft[:])
        octx.close()
        return

    # ---------------- o_proj half-B (heads 8..15) + merge ----------
    woB_pool = octx.enter_context(tc.tile_pool(name="woB", bufs=1))
    ost_pool = octx.enter_context(tc.tile_pool(name="ost", bufs=1))
    pb = octx.enter_context(tc.tile_pool(name="pb", bufs=1, space="PSUM"))
    woB = woB_pool.tile([128, 8 * HID], BF16)
    woB3 = woB[:].rearrange("p (j h) -> p j h", h=2048)
    worB = d["woT"][:].rearrange("(j p) h -> p j h", p=128)
    for w in range(4):
        nc.gpsimd.dma_start(out=woB3[:, w * 2:(w + 1) * 2, :],
                            in_=worB[:, (4 + w) * 2:(5 + w) * 2, :])
    for tile_idx in range(16):
        st, hc = divmod(tile_idx, 4)
        po = pb.tile([128, 512], F32, tag="po", bufs=2, name="poB")
        for j in range(8, NH):
            nc.tensor.matmul(
                po[:],
                AVT[:, j * SC + st * 128: j * SC + (st + 1) * 128],
                woB[:, (j - 8) * HID + hc * 512:
                    (j - 8) * HID + (hc + 1) * 512],
                start=(j == 8), stop=(j == NH - 1),
                skip_group_check=True)
        oct_ = ost_pool.tile([128, 512], F32, tag="oct", bufs=3,
                             name="oct")
        nc.vector.tensor_add(
            oct_[:], octA[:, tile_idx * 512:(tile_idx + 1) * 512],
            po[:])
        nc.scalar.dma_start(
            out=d["out"][st * 128:(st + 1) * 128,
                         hc * 512:(hc + 1) * 512],
            in_=oct_[:])
    octx.close()


def _emit_dma_only(nc, tc, mode, d, ropes, KTs, Vs):
    """DMA skeleton only (for phase timing)."""
    with contextlib.ExitStack() as ctx:
        hs_pool = ctx.enter_context(tc.tile_pool(name="hsT", bufs=1))
        wo_pool = ctx.enter_context(tc.tile_pool(name="wo", bufs=1))
        oacc_pool = ctx.enter_context(tc.tile_pool(name="oacc", bufs=1))

        hsT = hs_pool.tile([128, KT_H * SC], BF16)
        wk_full = hs_pool.tile([128, KT_H * NKV * D], BF16)
        wv_full = hs_pool.tile([128, KT_H * NKV * D], BF16)
        wq_full = hs_pool.tile([128, KT_H * NH * D], BF16)
        _emit_loads(nc, d, hsT, wk_full, wv_full, wq_full)

        nc.sync.dma_start(
            out=d["kv_local"][:, 0:512].rearrange("(kvh d) s -> d kvh s", d=128),
            in_=hsT[:, 0:2048].rearrange("d (kvh s) -> d kvh s", s=512))
        nc.scalar.dma_start(
            out=d["kv_local"][:, 512:1024].rearrange("(st p) c -> p st c", p=128),
            in_=hsT[:, 2048:4096].rearrange("p (st c) -> p st c", c=512))
        for r in range(4):
            nc.sync.dma_start(out=d["kv_all"][r * SC:(r + 1) * SC, :],
                              in_=d["kv_local"][:])
        nc.sync.dma_start(
            out=KTs[:].rearrange("d (r kvh s) -> d r kvh s", kvh=4, s=512),
            in_=d["kv_all"][:, 0:512].rearrange(
                "(r kvh d) s -> d r kvh s", r=4, kvh=4))
        nc.sync.dma_start(
            out=Vs[:].rearrange("p (r tt c) -> p r tt c", tt=4, c=512),
            in_=d["kv_all"][:, 512:1024].rearrange(
                "(r tt p) c -> p r tt c", r=4, tt=4))
        wo_full = wo_pool.tile([128, NH * HID], BF16)
        wor = d["woT"][:].rearrange("(j p) h -> p j h", p=128)
        wo3 = wo_full[:].rearrange("p (j h) -> p j h", h=2048)
        for w in range(8):
            nc.gpsimd.dma_start(out=wo3[:, w * 2:(w + 1) * 2, :],
                                in_=wor[:, w * 2:(w + 1) * 2, :])
        out_t = oacc_pool.tile([128, HID], F32)
        nc.vector.memset(out_t[:], 0.0)
        for st in range(4):
            nc.sync.dma_start(out=d["out"][st * 128:(st + 1) * 128, :],
                              in_=out_t[:])


def host_prep(hidden_states, cos, sin, Wq, Wk, Wv, Wo, q_norm_w, k_norm_w):
    """Build the 8 per-core input maps (host-side layout prep + bf16 cast)."""
    hs = np.asarray(hidden_states, dtype=np.float32)
    cos = np.asarray(cos, dtype=np.float32)
    sin = np.asarray(sin, dtype=np.float32)
    qw = np.asarray(q_norm_w, np.float32)
    kw = np.asarray(k_norm_w, np.float32)
    # signed sin (rotate_half), rolled by 64 so both DVE inputs share a
    # partition base; norm weights folded elementwise (position d of the
    # rolled table multiplies raw x[d], so the fold is just * w[d]).
    sinp = np.concatenate([-sin[..., :64], sin[..., 64:]], axis=-1)
    sinpsh = np.roll(sinp, -64, axis=-1)
    cosq = cos * qw
    sinqsh = sinpsh * qw
    cosk = cos * kw
    sinksh = sinpsh * kw
    wqT = np.ascontiguousarray(np.asarray(Wq, np.float32).T.astype(NP_BF16))
    wkT = np.ascontiguousarray(np.asarray(Wk, np.float32).T.astype(NP_BF16))
    wvT = np.ascontiguousarray(np.asarray(Wv, np.float32).T.astype(NP_BF16))
    woT = np.ascontiguousarray(np.asarray(Wo, np.float32).T.astype(NP_BF16))

    in_maps = []
    for core in range(8):
        b, sc = divmod(core, 4)
        sl = slice(sc * SC, (sc + 1) * SC)
        ropeT = np.concatenate(
            [cosq[b, sl].T, sinqsh[b, sl].T, cosk[b, sl].T, sinksh[b, sl].T],
            axis=1)
        in_maps.append({
            "xT": np.ascontiguousarray(hs[b, sl].T.astype(NP_BF16)),
            "wqT": wqT, "wkT": wkT, "wvT": wvT, "woT": woT,
            "ropeT": np.ascontiguousarray(ropeT.astype(NP_BF16)),
        })
    return in_maps


_nc_cache = {}


def get_nc(mode="real"):
    if mode not in _nc_cache:
        _nc_cache[mode] = build_nc(mode)
    return _nc_cache[mode]


def kernel(**inputs) -> np.ndarray:
    nc = get_nc("real")
    in_maps = host_prep(**inputs)
    res = run_bass_kernel_spmd(nc, in_maps, list(range(8)))
    out = np.empty((B, S, HID), np.float32)
    for core in range(8):
        b, sc = divmod(core, 4)
        out[b, sc * SC:(sc + 1) * SC, :] = res.results[core]["out"]
    return out


if __name__ == "__main__":
    import reference
    inputs = {k: np.asarray(v) for k, v in reference.setup_inputs().items()}
    expected = np.asarray(reference.reference(**inputs))
    actual = kernel(**inputs)
    err = np.abs(actual - expected)
    rel = err.max() / np.abs(expected).max()
    print(f"max abs err {err.max():.3e}  rel (vs absmax) {rel:.3e}")
